# revision 6
# baseline (speedup 1.0000x reference)
"""Trainium2 Bass kernel for causal multi-head attention (B=2, S=2048, D=1024, H=16).

Sharding: 32 (batch, head) pairs across 8 cores -> core c owns batch c//4,
heads 4*(c%4) .. 4*(c%4)+4.  Each core computes QKV projections for its head
slice, causal attention (probs written straight to its attn_prob slice), and
a partial output projection (its heads' rows of Wo).  Host sums the 4 partial
outputs per batch.

Matmuls run in bf16 (fast weight load + full PE rate); logits accumulate in
f32 PSUM.  Softmax skips the max-subtraction (logits are O(10); exp cannot
overflow in f32), letting ACT fuse exp + row-sum in one pass.  Logits are
computed in BOTH orientations: [sq, sk] for the attn_prob output (softmax
reduction along the free axis) and [sk, sq] to feed P@V without transposing
the 8.9M-element probability matrix (only the tiny 0.5M-element ctx^T is
transposed, twice, to normalize rows).  Strictly-upper-triangle blocks of
attn_prob are exactly zero (exp(-1e9) underflows) and are never computed or
written -- the output buffer is zero-initialized.

Bias handling (exact): bq folded into the Q copyback (per-partition scalar
add); bk shifts every logit in a row equally so softmax is invariant ->
dropped; bv and bo satisfy P @ (V + bv) @ Wo + bo = P@V@Wo + (bv@Wo + bo)
since P rows sum to 1 -> folded on the host.
"""

import ml_dtypes
import numpy as np

import concourse.mybir as mybir
import concourse.tile as tile
from concourse import bacc
from concourse.bass_utils import run_bass_kernel_spmd

F32 = mybir.dt.float32
BF16 = mybir.dt.bfloat16

B, S, D, H, DH = 2, 2048, 1024, 16, 64
NCORES = 8
HPC = H * B // NCORES  # heads per core = 4
CW = HPC * DH  # core's projection width = 256
KC = D // 128  # contraction chunks = 8
NQI = S // 128  # query 128-blocks = 16
NQS = S // 512  # query 512-supertiles = 4
EXP = mybir.ActivationFunctionType.Exp


def build(nc):
    xt = nc.dram_tensor("xt", [D, S], BF16, kind="ExternalInput").ap()
    wq = nc.dram_tensor("wq", [D, CW], BF16, kind="ExternalInput").ap()
    wk = nc.dram_tensor("wk", [D, CW], BF16, kind="ExternalInput").ap()
    wv = nc.dram_tensor("wv", [D, CW], BF16, kind="ExternalInput").ap()
    wo = nc.dram_tensor("wo", [CW, D], BF16, kind="ExternalInput").ap()
    bqs = nc.dram_tensor("bqs", [128, CW // 128], F32, kind="ExternalInput").ap()
    mask_nat = nc.dram_tensor("mask_nat", [128, 128], F32, kind="ExternalInput").ap()
    mask_tr = nc.dram_tensor("mask_tr", [128, 128], F32, kind="ExternalInput").ap()
    ident = nc.dram_tensor("ident", [128, 128], BF16, kind="ExternalInput").ap()

    p_out = nc.dram_tensor("p_out", [HPC, S, S], F32, kind="ExternalOutput").ap()
    o_partial = nc.dram_tensor("o_partial", [S, D], F32, kind="ExternalOutput").ap()

    xt_r = xt.rearrange("(c p) s -> p c s", p=128)
    wq_r = wq.rearrange("(c p) n -> p c n", p=128)
    wk_r = wk.rearrange("(c p) n -> p c n", p=128)
    wv_r = wv.rearrange("(c p) n -> p c n", p=128)
    wo_r = wo.rearrange("(c p) n -> p c n", p=128)

    with tile.TileContext(nc) as tc:
        with (
            tc.tile_pool(name="const", bufs=1) as cpool,
            tc.tile_pool(name="persist", bufs=1) as ppool,
        ):
            mn_sb = cpool.tile([128, 128], F32, tag="mn")
            mt_sb = cpool.tile([128, 128], F32, tag="mt")
            id_sb = cpool.tile([128, 128], BF16, tag="id")
            bq_sb = cpool.tile([128, CW // 128], F32, tag="bq")
            nc.sync.dma_start(mn_sb[:], mask_nat)
            nc.sync.dma_start(mt_sb[:], mask_tr)
            nc.sync.dma_start(id_sb[:], ident)
            nc.sync.dma_start(bq_sb[:], bqs)

            qt_sb = ppool.tile([128, 2, S], BF16, tag="qt")
            kt_sb = ppool.tile([128, 2, S], BF16, tag="kt")
            v_sb = ppool.tile([128, NQI, CW], BF16, tag="v")
            ctxt_sb = ppool.tile([128, 2, S], BF16, tag="ctxt")
            sums_sb = ppool.tile([128, HPC, NQI], F32, tag="sums")
            inv_sb = ppool.tile([128, HPC, NQI], F32, tag="inv")

            # ---------------- Phase B: QKV projections ----------------
            with (
                tc.tile_pool(name="xw", bufs=1) as xw,
                tc.tile_pool(name="projps", bufs=3, space="PSUM") as pps,
            ):
                xt_sb = xw.tile([128, KC, S], BF16, tag="xt")
                wq_sb = xw.tile([128, KC, CW], BF16, tag="wq")
                wk_sb = xw.tile([128, KC, CW], BF16, tag="wk")
                wv_sb = xw.tile([128, KC, CW], BF16, tag="wv")
                nc.sync.dma_start(xt_sb[:], xt_r)
                nc.sync.dma_start(wq_sb[:], wq_r)
                nc.sync.dma_start(wk_sb[:], wk_r)
                nc.sync.dma_start(wv_sb[:], wv_r)

                # Q^T and K^T: [256, S] as two 128-partition chunks
                for g in range(2):
                    for n in range(S // 512):
                        sl = slice(n * 512, (n + 1) * 512)
                        qp = pps.tile([128, 512], F32, tag="pj")
                        for kc in range(KC):
                            nc.tensor.matmul(
                                qp[:],
                                wq_sb[:, kc, g * 128 : (g + 1) * 128],
                                xt_sb[:, kc, sl],
                                start=(kc == 0),
                                stop=(kc == KC - 1),
                            )
                        # wq is pre-scaled by 1/8 on the host; add bq/8 here
                        nc.vector.tensor_scalar_add(
                            qt_sb[:, g, sl], qp[:], bq_sb[:, g : g + 1]
                        )
                        kp = pps.tile([128, 512], F32, tag="pj")
                        for kc in range(KC):
                            nc.tensor.matmul(
                                kp[:],
                                wk_sb[:, kc, g * 128 : (g + 1) * 128],
                                xt_sb[:, kc, sl],
                                start=(kc == 0),
                                stop=(kc == KC - 1),
                            )
                        nc.vector.tensor_copy(kt_sb[:, g, sl], kp[:])

                # V: [S, 256] as 16 row-tiles
                for st in range(NQI):
                    vp = pps.tile([128, CW], F32, tag="pj")
                    for kc in range(KC):
                        nc.tensor.matmul(
                            vp[:],
                            xt_sb[:, kc, st * 128 : (st + 1) * 128],
                            wv_sb[:, kc, :],
                            start=(kc == 0),
                            stop=(kc == KC - 1),
                        )
                    nc.vector.tensor_copy(v_sb[:, st, :], vp[:])

            # ---------------- Phase C: attention per head ----------------
            with (
                tc.tile_pool(name="cexp1", bufs=3) as e1pool,
                tc.tile_pool(name="cp1", bufs=3) as p1pool,
                tc.tile_pool(name="cparts", bufs=4) as partpool,
                tc.tile_pool(name="cexp2", bufs=6) as e2pool,
                tc.tile_pool(name="cctxs", bufs=2) as crawpool,
                tc.tile_pool(name="log1ps", bufs=2, space="PSUM") as log1ps,
                tc.tile_pool(name="log2ps", bufs=3, space="PSUM") as log2ps,
                tc.tile_pool(name="ctxps", bufs=1, space="PSUM") as ctxps,
            ):
                for h in range(HPC):
                    g = h // 2
                    po = 64 * (h % 2)
                    qt_h = qt_sb[po : po + 64, g, :]
                    kt_h = kt_sb[po : po + 64, g, :]

                    for Qi in range(NQS):
                        # --- natural orientation: probs to DRAM ---
                        for qi in range(4 * Qi, 4 * Qi + 4):
                            W = (qi + 1) * 128
                            nch = (W + 1023) // 1024
                            exp1 = e1pool.tile([128, S], F32, tag="e1")
                            p1 = p1pool.tile([128, S], F32, tag="p1")
                            parts = partpool.tile([128, 2], F32, tag="pt")
                            for c in range(nch):
                                w = min(1024, W - c * 1024)
                                lp = log1ps.tile([128, 1024], F32, tag="l1")
                                for u in range(0, w, 512):
                                    uw = min(512, w - u)
                                    nc.tensor.matmul(
                                        lp[:, u : u + uw],
                                        qt_h[:, qi * 128 : (qi + 1) * 128],
                                        kt_h[:, c * 1024 + u : c * 1024 + u + uw],
                                        start=True,
                                        stop=True,
                                    )
                                if c == nch - 1:
                                    nc.vector.tensor_add(
                                        lp[:, w - 128 : w], lp[:, w - 128 : w], mn_sb[:]
                                    )
                                nc.scalar.activation(
                                    exp1[:, c * 1024 : c * 1024 + w],
                                    lp[:, :w],
                                    EXP,
                                    accum_out=parts[:, c : c + 1],
                                )
                            nc.vector.reduce_sum(
                                sums_sb[:, h, qi : qi + 1],
                                parts[:, :nch],
                                axis=mybir.AxisListType.X,
                            )
                            nc.vector.reciprocal(
                                inv_sb[:, h, qi : qi + 1], sums_sb[:, h, qi : qi + 1]
                            )
                            nc.vector.tensor_scalar_mul(
                                p1[:, :W], exp1[:, :W], inv_sb[:, h, qi : qi + 1]
                            )
                            nc.sync.dma_start(
                                p_out[h, qi * 128 : (qi + 1) * 128, 0:W], p1[:, :W]
                            )

                        # --- transposed orientation feeds PV ---
                        ctile = ctxps.tile([64, 512], F32, tag="cx", name=f"cx{h}_{Qi}")
                        for j in range(4 * Qi + 4):
                            vj = v_sb[:, j, h * 64 : (h + 1) * 64]
                            r0 = j - 4 * Qi if j // 4 == Qi else 0
                            off = r0 * 128
                            lt = log2ps.tile([128, 512], F32, tag="l2")
                            nc.tensor.matmul(
                                lt[:, off:512],
                                kt_h[:, j * 128 : (j + 1) * 128],
                                qt_h[:, Qi * 512 + off : (Qi + 1) * 512],
                                start=True,
                                stop=True,
                            )
                            if j // 4 == Qi:
                                nc.vector.tensor_add(
                                    lt[:, off : off + 128],
                                    lt[:, off : off + 128],
                                    mt_sb[:],
                                )
                            e2 = e2pool.tile([128, 512], BF16, tag="e2")
                            nc.scalar.activation(e2[:, off:512], lt[:, off:512], EXP)
                            nc.tensor.matmul(
                                ctile[:, off:512],
                                vj,
                                e2[:, off:512],
                                start=(j == 0),
                                stop=(j == 4 * Qi + 3),
                            )

                        # normalize ctx^T rows via two small transposes
                        craw = crawpool.tile([64, 512], BF16, tag="cr")
                        nc.vector.tensor_copy(craw[:], ctile[:])
                        for t in range(4):
                            qi = Qi * 4 + t
                            tp1 = log2ps.tile([128, 64], BF16, tag="l2")
                            nc.tensor.transpose(
                                tp1[:],
                                craw[:, t * 128 : (t + 1) * 128],
                                id_sb[0:64, 0:64],
                            )
                            cnat = crawpool.tile([128, 64], BF16, tag="cn")
                            nc.vector.tensor_scalar_mul(
                                cnat[:], tp1[:], inv_sb[:, h, qi : qi + 1]
                            )
                            tp2 = log2ps.tile([64, 128], BF16, tag="l2")
                            nc.tensor.transpose(tp2[:], cnat[:], id_sb[:])
                            nc.vector.tensor_copy(
                                ctxt_sb[po : po + 64, g, qi * 128 : (qi + 1) * 128],
                                tp2[:],
                            )

            # ---------------- Phase D: output projection ----------------
            with (
                tc.tile_pool(name="dwo", bufs=1) as wopool,
                tc.tile_pool(name="dout", bufs=3) as opool,
                tc.tile_pool(name="outps", bufs=3, space="PSUM") as ops,
            ):
                wo_sb = wopool.tile([128, 2, D], BF16, tag="wo")
                nc.sync.dma_start(wo_sb[:], wo_r)
                for st in range(NQI):
                    for n in range(2):
                        op = ops.tile([128, 512], F32, tag="op")
                        for kc in range(2):
                            nc.tensor.matmul(
                                op[:],
                                ctxt_sb[:, kc, st * 128 : (st + 1) * 128],
                                wo_sb[:, kc, n * 512 : (n + 1) * 512],
                                start=(kc == 0),
                                stop=(kc == 1),
                            )
                        ob = opool.tile([128, 512], F32, tag="ob")
                        nc.vector.tensor_copy(ob[:], op[:])
                        nc.sync.dma_start(
                            o_partial[
                                st * 128 : (st + 1) * 128, n * 512 : (n + 1) * 512
                            ],
                            ob[:],
                        )
    return nc


_COMPILED = None


def _get_compiled():
    global _COMPILED
    if _COMPILED is None:
        nc = bacc.Bacc(
            "TRN2", target_bir_lowering=False, debug=False, num_devices=NCORES
        )
        build(nc)
        nc.compile()
        _COMPILED = nc
    return _COMPILED


def _numpy_reference(inputs, mask, Wq, bq, Wk, bk, Wv, bv, Wo, bo):
    """Fallback for a non-causal mask (never hit for this problem's inputs)."""
    x = inputs.astype(np.float64)
    q = (x @ Wq.astype(np.float64) + bq).reshape(B, S, H, DH).transpose(0, 2, 1, 3)
    k = (x @ Wk.astype(np.float64) + bk).reshape(B, S, H, DH).transpose(0, 2, 1, 3)
    v = (x @ Wv.astype(np.float64) + bv).reshape(B, S, H, DH).transpose(0, 2, 1, 3)
    logits = np.einsum("bhqd,bhkd->bhqk", q, k) / np.sqrt(DH)
    logits = logits + mask.astype(np.float64) * -1e9
    logits -= logits.max(-1, keepdims=True)
    p = np.exp(logits)
    p /= p.sum(-1, keepdims=True)
    ctx = np.einsum("bhqk,bhkd->bhqd", p, v)
    ctx = ctx.transpose(0, 2, 1, 3).reshape(B, S, D)
    out = ctx @ Wo.astype(np.float64) + bo
    return out.astype(np.float32), p.astype(np.float32)


def _build_in_maps(inputs, Wq, bq, Wk, Wv, Wo):
    mask_nat = np.where(np.triu(np.ones((128, 128), bool), 1), -1e9, 0.0).astype(
        np.float32
    )
    mask_tr = np.ascontiguousarray(mask_nat.T)
    ident = np.eye(128, dtype=ml_dtypes.bfloat16)
    bf = ml_dtypes.bfloat16

    in_maps = []
    for c in range(NCORES):
        b = c // (NCORES // B)
        h0 = (c % (NCORES // B)) * HPC
        cols = slice(h0 * DH, (h0 + HPC) * DH)
        bq_s = (bq[cols].astype(np.float32) * 0.125).reshape(CW // 128, 128).T
        in_maps.append(
            {
                "xt": np.ascontiguousarray(inputs[b].T).astype(bf),
                "wq": (Wq[:, cols] * 0.125).astype(bf),
                "wk": np.ascontiguousarray(Wk[:, cols]).astype(bf),
                "wv": np.ascontiguousarray(Wv[:, cols]).astype(bf),
                "wo": np.ascontiguousarray(Wo[cols, :]).astype(bf),
                "bqs": np.ascontiguousarray(bq_s),
                "mask_nat": mask_nat,
                "mask_tr": mask_tr,
                "ident": ident,
            }
        )
    return in_maps


def kernel(inputs, mask, Wq, bq, Wk, bk, Wv, bv, Wo, bo):
    inputs = np.asarray(inputs, dtype=np.float32)
    mask = np.asarray(mask, dtype=np.float32)
    Wq, bq = np.asarray(Wq, np.float32), np.asarray(bq, np.float32)
    Wk, bk = np.asarray(Wk, np.float32), np.asarray(bk, np.float32)
    Wv, bv = np.asarray(Wv, np.float32), np.asarray(bv, np.float32)
    Wo, bo = np.asarray(Wo, np.float32), np.asarray(bo, np.float32)

    causal = np.triu(np.ones((S, S), dtype=np.float32), k=1)
    if not np.array_equal(mask, np.broadcast_to(causal[None, None], mask.shape)):
        return _numpy_reference(inputs, mask, Wq, bq, Wk, bk, Wv, bv, Wo, bo)

    in_maps = _build_in_maps(inputs, Wq, bq, Wk, Wv, Wo)
    nc = _get_compiled()
    res = run_bass_kernel_spmd(nc, in_maps, core_ids=list(range(NCORES)))

    attn = np.empty((B, H, S, S), dtype=np.float32)
    out = np.zeros((B, S, D), dtype=np.float32)
    for c in range(NCORES):
        b = c // (NCORES // B)
        h0 = (c % (NCORES // B)) * HPC
        attn[b, h0 : h0 + HPC] = res.results[c]["p_out"]
        out[b] += res.results[c]["o_partial"]
    out += (bv @ Wo + bo)[None, None, :]
    return out, attn


# revision 9
# speedup vs baseline: 1.0951x; 1.0951x over previous
"""Trainium2 Bass kernel for causal multi-head attention (B=2, S=2048, D=1024, H=16).

Sharding: 32 (batch, head) pairs across 8 cores -> core c owns batch c//4,
heads 4*(c%4) .. 4*(c%4)+4.  Each core computes QKV projections for its head
slice, causal attention (probs written straight to its attn_prob slice), and
a partial output projection (its heads' rows of Wo).  Host sums the 4 partial
outputs per batch.

Matmuls run in bf16 (fast weight load + full PE rate); logits accumulate in
f32 PSUM.  Softmax skips the max-subtraction (logits are O(10); exp cannot
overflow in f32), letting ACT fuse exp + row-sum in one pass.  Logits are
computed in BOTH orientations: [sq, sk] for the attn_prob output (softmax
reduction along the free axis) and [sk, sq] to feed P@V without transposing
the 8.9M-element probability matrix (only the tiny 0.5M-element ctx^T is
transposed, twice, to normalize rows).  Strictly-upper-triangle blocks of
attn_prob are exactly zero (exp(-1e9) underflows) and are never computed or
written -- the output buffer is zero-initialized.

Bias handling (exact): bq folded into the Q copyback (per-partition scalar
add); bk shifts every logit in a row equally so softmax is invariant ->
dropped; bv and bo satisfy P @ (V + bv) @ Wo + bo = P@V@Wo + (bv@Wo + bo)
since P rows sum to 1 -> folded on the host.
"""

import ml_dtypes
import numpy as np

import concourse.library_config as library_config
import concourse.mybir as mybir
import concourse.tile as tile
from concourse import bacc
from concourse.bass_utils import run_bass_kernel_spmd

F32 = mybir.dt.float32
BF16 = mybir.dt.bfloat16

B, S, D, H, DH = 2, 2048, 1024, 16, 64
NCORES = 8
HPC = H * B // NCORES  # heads per core = 4
CW = HPC * DH  # core's projection width = 256
KC = D // 128  # contraction chunks = 8
NQI = S // 128  # query 128-blocks = 16
NQS = S // 512  # query 512-supertiles = 4
EXP = mybir.ActivationFunctionType.Exp


def build(nc):
    xt = nc.dram_tensor("xt", [D, S], BF16, kind="ExternalInput").ap()
    wq = nc.dram_tensor("wq", [D, CW], BF16, kind="ExternalInput").ap()
    wk = nc.dram_tensor("wk", [D, CW], BF16, kind="ExternalInput").ap()
    wv = nc.dram_tensor("wv", [D, CW], BF16, kind="ExternalInput").ap()
    wo = nc.dram_tensor("wo", [CW, D], BF16, kind="ExternalInput").ap()
    bqs = nc.dram_tensor("bqs", [128, CW // 128], F32, kind="ExternalInput").ap()
    mask_nat = nc.dram_tensor("mask_nat", [128, 128], F32, kind="ExternalInput").ap()
    mask_tr = nc.dram_tensor("mask_tr", [128, 128], F32, kind="ExternalInput").ap()
    ident = nc.dram_tensor("ident", [128, 128], BF16, kind="ExternalInput").ap()

    p_out = nc.dram_tensor("p_out", [HPC, S, S], F32, kind="ExternalOutput").ap()
    o_partial = nc.dram_tensor("o_partial", [S, D], F32, kind="ExternalOutput").ap()

    xt_r = xt.rearrange("(c p) s -> p c s", p=128)
    wq_r = wq.rearrange("(c p) n -> p c n", p=128)
    wk_r = wk.rearrange("(c p) n -> p c n", p=128)
    wv_r = wv.rearrange("(c p) n -> p c n", p=128)
    wo_r = wo.rearrange("(c p) n -> p c n", p=128)

    with tile.TileContext(nc) as tc:
        with (
            tc.tile_pool(name="const", bufs=1) as cpool,
            tc.tile_pool(name="persist", bufs=1) as ppool,
        ):
            mn_sb = cpool.tile([128, 128], F32, tag="mn")
            mt_sb = cpool.tile([128, 128], F32, tag="mt")
            id_sb = cpool.tile([128, 128], BF16, tag="id")
            bq_sb = cpool.tile([128, CW // 128], F32, tag="bq")
            nc.sync.dma_start(mn_sb[:], mask_nat)
            nc.sync.dma_start(mt_sb[:], mask_tr)
            nc.sync.dma_start(id_sb[:], ident)
            nc.sync.dma_start(bq_sb[:], bqs)

            qt_sb = ppool.tile([128, 2, S], BF16, tag="qt")
            kt_sb = ppool.tile([128, 2, S], BF16, tag="kt")
            v_sb = ppool.tile([128, NQI, CW], BF16, tag="v")
            ctxt_sb = ppool.tile([128, 2, S], BF16, tag="ctxt")
            sums_sb = ppool.tile([128, HPC, NQI], F32, tag="sums")
            inv_sb = ppool.tile([128, HPC, NQI], F32, tag="inv")

            # ---------------- Phase B: QKV projections ----------------
            with (
                tc.tile_pool(name="xw", bufs=1) as xw,
                tc.tile_pool(name="projps", bufs=3, space="PSUM") as pps,
            ):
                xt_sb = xw.tile([128, KC, S], BF16, tag="xt")
                wq_sb = xw.tile([128, KC, CW], BF16, tag="wq")
                wk_sb = xw.tile([128, KC, CW], BF16, tag="wk")
                wv_sb = xw.tile([128, KC, CW], BF16, tag="wv")
                nc.sync.dma_start(xt_sb[:], xt_r)
                nc.sync.dma_start(wq_sb[:], wq_r)
                nc.sync.dma_start(wk_sb[:], wk_r)
                nc.sync.dma_start(wv_sb[:], wv_r)

                # Q^T and K^T: [256, S] as two 128-partition chunks
                for g in range(2):
                    for n in range(S // 512):
                        sl = slice(n * 512, (n + 1) * 512)
                        qp = pps.tile([128, 512], F32, tag="pj")
                        for kc in range(KC):
                            nc.tensor.matmul(
                                qp[:],
                                wq_sb[:, kc, g * 128 : (g + 1) * 128],
                                xt_sb[:, kc, sl],
                                start=(kc == 0),
                                stop=(kc == KC - 1),
                            )
                        # wq is pre-scaled by 1/8 on the host; add bq/8 here
                        nc.vector.tensor_scalar_add(
                            qt_sb[:, g, sl], qp[:], bq_sb[:, g : g + 1]
                        )
                        kp = pps.tile([128, 512], F32, tag="pj")
                        for kc in range(KC):
                            nc.tensor.matmul(
                                kp[:],
                                wk_sb[:, kc, g * 128 : (g + 1) * 128],
                                xt_sb[:, kc, sl],
                                start=(kc == 0),
                                stop=(kc == KC - 1),
                            )
                        nc.vector.tensor_copy(kt_sb[:, g, sl], kp[:])

                # V: [S, 256] as 16 row-tiles
                for st in range(NQI):
                    vp = pps.tile([128, CW], F32, tag="pj")
                    for kc in range(KC):
                        nc.tensor.matmul(
                            vp[:],
                            xt_sb[:, kc, st * 128 : (st + 1) * 128],
                            wv_sb[:, kc, :],
                            start=(kc == 0),
                            stop=(kc == KC - 1),
                        )
                    nc.vector.tensor_copy(v_sb[:, st, :], vp[:])

            # ---------------- Phase C: attention per head ----------------
            nc.gpsimd.load_library(library_config.attn)
            with (
                tc.tile_pool(name="cexp1", bufs=3) as e1pool,
                tc.tile_pool(name="cp1", bufs=3) as p1pool,
                tc.tile_pool(name="cparts", bufs=4) as partpool,
                tc.tile_pool(name="cexp2", bufs=18) as e2pool,
                tc.tile_pool(name="cinv", bufs=3) as invpool,
                tc.tile_pool(name="cbc", bufs=2) as bcpool,
                tc.tile_pool(name="cdr", bufs=2, space="DRAM") as drampool,
                tc.tile_pool(name="log1ps", bufs=2, space="PSUM") as log1ps,
                tc.tile_pool(name="log2ps", bufs=3, space="PSUM") as log2ps,
                tc.tile_pool(name="ctxps", bufs=1, space="PSUM") as ctxps,
            ):
                for h in range(HPC):
                    g = h // 2
                    po = 64 * (h % 2)
                    qt_h = qt_sb[po : po + 64, g, :]
                    kt_h = kt_sb[po : po + 64, g, :]

                    for Qi in range(NQS):
                        # --- natural orientation: probs to DRAM ---
                        for qi in range(4 * Qi, 4 * Qi + 4):
                            W = (qi + 1) * 128
                            nch = (W + 1023) // 1024
                            exp1 = e1pool.tile([128, S], F32, tag="e1")
                            p1 = p1pool.tile([128, S], F32, tag="p1")
                            parts = partpool.tile([128, 2], F32, tag="pt")
                            for c in range(nch):
                                w = min(1024, W - c * 1024)
                                lp = log1ps.tile([128, 1024], F32, tag="l1")
                                for u in range(0, w, 512):
                                    uw = min(512, w - u)
                                    nc.tensor.matmul(
                                        lp[:, u : u + uw],
                                        qt_h[:, qi * 128 : (qi + 1) * 128],
                                        kt_h[:, c * 1024 + u : c * 1024 + u + uw],
                                        start=True,
                                        stop=True,
                                    )
                                if c == nch - 1:
                                    nc.vector.tensor_add(
                                        lp[:, w - 128 : w], lp[:, w - 128 : w], mn_sb[:]
                                    )
                                nc.scalar.activation(
                                    exp1[:, c * 1024 : c * 1024 + w],
                                    lp[:, :w],
                                    EXP,
                                    accum_out=parts[:, c : c + 1],
                                )
                            nc.vector.reduce_sum(
                                sums_sb[:, h, qi : qi + 1],
                                parts[:, :nch],
                                axis=mybir.AxisListType.X,
                            )
                            nc.vector.reciprocal(
                                inv_sb[:, h, qi : qi + 1], sums_sb[:, h, qi : qi + 1]
                            )
                            nc.vector.tensor_scalar_mul(
                                p1[:, :W], exp1[:, :W], inv_sb[:, h, qi : qi + 1]
                            )
                            nc.sync.dma_start(
                                p_out[h, qi * 128 : (qi + 1) * 128, 0:W], p1[:, :W]
                            )

                        # --- transposed orientation: logits^T -> exp^T ---
                        e2_tiles = []
                        for j in range(4 * Qi + 4):
                            r0 = j - 4 * Qi if j // 4 == Qi else 0
                            off = r0 * 128
                            lt = log2ps.tile([128, 512], F32, tag="l2")
                            nc.tensor.matmul(
                                lt[:, off:512],
                                kt_h[:, j * 128 : (j + 1) * 128],
                                qt_h[:, Qi * 512 + off : (Qi + 1) * 512],
                                start=True,
                                stop=True,
                            )
                            if j // 4 == Qi:
                                nc.vector.tensor_add(
                                    lt[:, off : off + 128],
                                    lt[:, off : off + 128],
                                    mt_sb[:],
                                )
                            e2 = e2pool.tile(
                                [128, 512], BF16, tag="e2", name=f"e2_{h}_{Qi}_{j}"
                            )
                            nc.scalar.activation(e2[:, off:512], lt[:, off:512], EXP)
                            e2_tiles.append((e2, off))

                        # --- PV: all weights/operands ready, pure PE stream ---
                        ctile = ctxps.tile([64, 512], F32, tag="cx", name=f"cx{h}_{Qi}")
                        for j in range(4 * Qi + 4):
                            e2, off = e2_tiles[j]
                            nc.tensor.matmul(
                                ctile[:, off:512],
                                v_sb[:, j, h * 64 : (h + 1) * 64],
                                e2[:, off:512],
                                start=(j == 0),
                                stop=(j == 4 * Qi + 3),
                            )

                        # normalize ctx^T rows: broadcast 1/rowsum along sq
                        invb = invpool.tile([128, 4], BF16, tag="invb")
                        nc.vector.tensor_copy(invb[:], inv_sb[:, h, 4 * Qi : 4 * Qi + 4])
                        tpr = log2ps.tile([4, 128], BF16, tag="l2")
                        nc.tensor.transpose(tpr[:], invb[:], id_sb[:])
                        invrow4 = invpool.tile([4, 128], BF16, tag="ir4")
                        nc.vector.tensor_copy(invrow4[:], tpr[:])
                        dr = drampool.tile([1, 512], BF16, tag="dr")
                        nc.sync.dma_start(
                            dr[:].rearrange("a (b c) -> (a b) c", b=4), invrow4[:]
                        )
                        invrow = invpool.tile([1, 512], BF16, tag="ir1")
                        nc.sync.dma_start(invrow[:], dr[:])
                        bc = bcpool.tile([64, 512], BF16, tag="bc")
                        nc.gpsimd.partition_broadcast(bc[:], invrow[:])
                        nc.vector.tensor_mul(
                            ctxt_sb[po : po + 64, g, Qi * 512 : (Qi + 1) * 512],
                            ctile[:],
                            bc[:],
                        )

            # ---------------- Phase D: output projection ----------------
            with (
                tc.tile_pool(name="dwo", bufs=1) as wopool,
                tc.tile_pool(name="dout", bufs=3) as opool,
                tc.tile_pool(name="outps", bufs=3, space="PSUM") as ops,
            ):
                wo_sb = wopool.tile([128, 2, D], BF16, tag="wo")
                nc.sync.dma_start(wo_sb[:], wo_r)
                for st in range(NQI):
                    for n in range(2):
                        op = ops.tile([128, 512], F32, tag="op")
                        for kc in range(2):
                            nc.tensor.matmul(
                                op[:],
                                ctxt_sb[:, kc, st * 128 : (st + 1) * 128],
                                wo_sb[:, kc, n * 512 : (n + 1) * 512],
                                start=(kc == 0),
                                stop=(kc == 1),
                            )
                        ob = opool.tile([128, 512], F32, tag="ob")
                        nc.vector.tensor_copy(ob[:], op[:])
                        nc.sync.dma_start(
                            o_partial[
                                st * 128 : (st + 1) * 128, n * 512 : (n + 1) * 512
                            ],
                            ob[:],
                        )
    return nc


_COMPILED = None


def _get_compiled():
    global _COMPILED
    if _COMPILED is None:
        nc = bacc.Bacc(
            "TRN2", target_bir_lowering=False, debug=False, num_devices=NCORES
        )
        build(nc)
        nc.compile()
        _COMPILED = nc
    return _COMPILED


def _numpy_reference(inputs, mask, Wq, bq, Wk, bk, Wv, bv, Wo, bo):
    """Fallback for a non-causal mask (never hit for this problem's inputs)."""
    x = inputs.astype(np.float64)
    q = (x @ Wq.astype(np.float64) + bq).reshape(B, S, H, DH).transpose(0, 2, 1, 3)
    k = (x @ Wk.astype(np.float64) + bk).reshape(B, S, H, DH).transpose(0, 2, 1, 3)
    v = (x @ Wv.astype(np.float64) + bv).reshape(B, S, H, DH).transpose(0, 2, 1, 3)
    logits = np.einsum("bhqd,bhkd->bhqk", q, k) / np.sqrt(DH)
    logits = logits + mask.astype(np.float64) * -1e9
    logits -= logits.max(-1, keepdims=True)
    p = np.exp(logits)
    p /= p.sum(-1, keepdims=True)
    ctx = np.einsum("bhqk,bhkd->bhqd", p, v)
    ctx = ctx.transpose(0, 2, 1, 3).reshape(B, S, D)
    out = ctx @ Wo.astype(np.float64) + bo
    return out.astype(np.float32), p.astype(np.float32)


def _build_in_maps(inputs, Wq, bq, Wk, Wv, Wo):
    mask_nat = np.where(np.triu(np.ones((128, 128), bool), 1), -1e9, 0.0).astype(
        np.float32
    )
    mask_tr = np.ascontiguousarray(mask_nat.T)
    ident = np.eye(128, dtype=ml_dtypes.bfloat16)
    bf = ml_dtypes.bfloat16

    in_maps = []
    for c in range(NCORES):
        b = c // (NCORES // B)
        h0 = (c % (NCORES // B)) * HPC
        cols = slice(h0 * DH, (h0 + HPC) * DH)
        bq_s = (bq[cols].astype(np.float32) * 0.125).reshape(CW // 128, 128).T
        in_maps.append(
            {
                "xt": np.ascontiguousarray(inputs[b].T).astype(bf),
                "wq": (Wq[:, cols] * 0.125).astype(bf),
                "wk": np.ascontiguousarray(Wk[:, cols]).astype(bf),
                "wv": np.ascontiguousarray(Wv[:, cols]).astype(bf),
                "wo": np.ascontiguousarray(Wo[cols, :]).astype(bf),
                "bqs": np.ascontiguousarray(bq_s),
                "mask_nat": mask_nat,
                "mask_tr": mask_tr,
                "ident": ident,
            }
        )
    return in_maps


def kernel(inputs, mask, Wq, bq, Wk, bk, Wv, bv, Wo, bo):
    inputs = np.asarray(inputs, dtype=np.float32)
    mask = np.asarray(mask, dtype=np.float32)
    Wq, bq = np.asarray(Wq, np.float32), np.asarray(bq, np.float32)
    Wk, bk = np.asarray(Wk, np.float32), np.asarray(bk, np.float32)
    Wv, bv = np.asarray(Wv, np.float32), np.asarray(bv, np.float32)
    Wo, bo = np.asarray(Wo, np.float32), np.asarray(bo, np.float32)

    causal = np.triu(np.ones((S, S), dtype=np.float32), k=1)
    if not np.array_equal(mask, np.broadcast_to(causal[None, None], mask.shape)):
        return _numpy_reference(inputs, mask, Wq, bq, Wk, bk, Wv, bv, Wo, bo)

    in_maps = _build_in_maps(inputs, Wq, bq, Wk, Wv, Wo)
    nc = _get_compiled()
    res = run_bass_kernel_spmd(nc, in_maps, core_ids=list(range(NCORES)))

    attn = np.empty((B, H, S, S), dtype=np.float32)
    out = np.zeros((B, S, D), dtype=np.float32)
    for c in range(NCORES):
        b = c // (NCORES // B)
        h0 = (c % (NCORES // B)) * HPC
        attn[b, h0 : h0 + HPC] = res.results[c]["p_out"]
        out[b] += res.results[c]["o_partial"]
    out += (bv @ Wo + bo)[None, None, :]
    return out, attn
